# revision 40
# baseline (speedup 1.0000x reference)
"""Trainium2 Bass kernel for nn_MemTransformerLM (DPFP linear-attention block).

Full inputs in, full output out. Internally: head-shards across 8 NeuronCores
(2 heads/core), runs causal linear attention as a chunked prefix-sum (the
reference's sum-normalized kernelized attention factorizes: no SxS score
materialization), AllToAll re-shards heads->rows for the output projection,
and each core LayerNorms its row slice. Host concatenates the 8 row slices.

Overlap structure: chunk columns are stored (cl, batch)-interleaved so the
DPFP products and the attention loop start halfway through the projections;
the attention runs head 0 fully, launches its AllToAll, and hides it behind
head 1's attention pass.
"""
import os
import sys
import types
from contextlib import ExitStack

for _p in ("/opt/trn_rl_repo",):
    if _p not in sys.path:
        sys.path.insert(0, _p)

import numpy as np
import ml_dtypes

import concourse.bass as bass
import concourse.mybir as mybir
import concourse.tile as tile
from concourse import bacc
from concourse.bass_utils import run_bass_kernel_spmd

BF16 = ml_dtypes.bfloat16
FP8 = ml_dtypes.float8_e4m3
F32 = np.float32

SEQ, BATCH, D = 1536, 2, 1024
NH, DH, NR = 16, 64, 3
SCALE = 1.0 / float(np.sqrt(DH))
S_FOLD = float(np.sqrt(SCALE))           # folded into Wq rows (squared by DPFP products)
EPS_D, EPS_LN = 1e-5, 1e-5
N_CORES = 8
HPC = NH // N_CORES                      # heads per core (2)
ROWS = SEQ * BATCH                       # 3072 batch-major rows
RPC = ROWS // N_CORES                    # 384 output rows per core
NCHUNK = ROWS // 128                     # 24 chunks of 128 rows
NCB = NCHUNK // BATCH                    # 12 chunks per batch
FEAT = 2 * DH * NR                       # 384 DPFP features
NKD = D // 128                           # 8 contraction chunks over d_model
PW = 3 * HPC * DH                        # 384 projection width (q|k|v)

dt = mybir.dt

# chunk storage position: pos = cl*2 + b  (global chunk c = b*NCB + cl)
POS_OF_C = [(c % NCB) * 2 + (c // NCB) for c in range(NCHUNK)]
C_OF_POS = [0] * NCHUNK
for _c, _p in enumerate(POS_OF_C):
    C_OF_POS[_p] = _c


def _install_profshim():
    """Enable NTFF profiling under axon when antenv.axon_hooks is missing."""
    try:
        import antenv
    except ImportError:
        return
    if "antenv.axon_hooks" in sys.modules:
        return
    mod = types.ModuleType("antenv.axon_hooks")
    mod._hook = None
    mod.set_axon_ntff_profile_hook = lambda h: setattr(mod, "_hook", h)
    mod.get_axon_ntff_profile_hook = lambda: mod._hook
    sys.modules["antenv.axon_hooks"] = mod
    antenv.axon_hooks = mod
    try:
        from trn_agent_boot.trn_boot import _ntff_profile_via_ctypes
        mod.set_axon_ntff_profile_hook(
            _ntff_profile_via_ctypes("/opt/axon/libaxon_pjrt.so"))
    except Exception:
        pass


def build_program():
    nc = bacc.Bacc("TRN2", target_bir_lowering=False, debug=False,
                   num_devices=N_CORES)

    # ---- kernel I/O (per-core values supplied via in_maps) ----
    hT_d = nc.declare_dram_parameter("hT", [NKD // 2, 128, 2 * ROWS],
                                     dt.float8e4, isOutput=False)
    wall_d = nc.declare_dram_parameter("wallT", [128, NKD * PW], dt.float8e4,
                                       isOutput=False)
    woT_d = nc.declare_dram_parameter("woT", [128, NKD * D], dt.bfloat16,
                                      isOutput=False)
    hs_d = nc.declare_dram_parameter("h_slice", [RPC, D], dt.float32, isOutput=False)
    mask_d = nc.declare_dram_parameter("mask4", [128, 512], dt.bfloat16, isOutput=False)
    ident_d = nc.declare_dram_parameter("ident", [128, 128], dt.bfloat16, isOutput=False)
    gamb_d = nc.declare_dram_parameter("gamma_bc", [128, D], dt.bfloat16, isOutput=False)
    betb_d = nc.declare_dram_parameter("beta_bc", [128, D], dt.bfloat16, isOutput=False)
    out_d = nc.declare_dram_parameter("out", [RPC, D], dt.bfloat16, isOutput=True)

    # internal DRAM bounce buffers: one combined AllToAll for both heads
    # (bf16: an fp8 payload corrupted 3/4 of the bytes and only saved ~2.5us
    # -- the 8-core A2A is latency-floor-bound, not bandwidth-bound)
    a2a_in = nc.dram_tensor("a2a_in", [N_CORES, HPC * DH, RPC], dt.bfloat16)
    a2a_out = nc.dram_tensor("a2a_out", [N_CORES, HPC * DH, RPC], dt.bfloat16)
    # tiny warmup collective: absorbs collective-stack cold-start / core skew
    # while phases 1-3 compute (no data deps)
    warm_in = nc.dram_tensor("warm_in", [N_CORES, 1, 64], dt.bfloat16)
    warm_out = nc.dram_tensor("warm_out", [N_CORES, 1, 64], dt.bfloat16)

    with tile.TileContext(nc) as tc:
        with (
            tc.tile_pool(name="const", bufs=1) as Pc,
            tc.tile_pool(name="big", bufs=1) as Pb,
            tc.tile_pool(name="work", bufs=2) as Pw,
            ExitStack() as _stack,
        ):
            _inner = ExitStack()
            Pi = _inner.enter_context(tc.tile_pool(name="inner", bufs=1))
            _ps2 = ExitStack()
            Pp = _ps2.enter_context(tc.tile_pool(name="ps2", bufs=2, space="PSUM"))
            _ps3 = ExitStack()
            Pp3 = _ps3.enter_context(tc.tile_pool(name="ps3", bufs=1, space="PSUM"))

            # warmup collective first: starts the CC handshake immediately so
            # the real A2As later see an already-synced collective stack
            nc.gpsimd.collective_compute(
                "AllToAll", mybir.AluOpType.bypass,
                replica_groups=[list(range(N_CORES))],
                ins=[warm_in.ap().opt()], outs=[warm_out.ap().opt()])

            # ---------- constants ----------
            mask4 = Pc.tile([128, 512], dt.bfloat16, tag="mask4")
            ident = Pc.tile([128, 128], dt.bfloat16, tag="ident")
            eps_ln = Pc.tile([128, 1], dt.float32, tag="eps_ln")
            nc.vector.memset(eps_ln[:, :], EPS_LN)
            nc.sync.dma_start(mask4[:, :], mask_d[:, :])
            nc.sync.dma_start(ident[:, :], ident_d[:, :])
            # gamma/beta arrive pre-broadcast from the host
            gam_bc = Pc.tile([128, D], dt.bfloat16, tag="gam_bc")
            bet_bc = Pc.tile([128, D], dt.bfloat16, tag="bet_bc")
            nc.sync.dma_start(gam_bc[:, :], gamb_d[:, :])
            nc.sync.dma_start(bet_bc[:, :], betb_d[:, :])

            # PE clock warmup while the first input DMAs are in flight
            warm_ps = Pp3.tile([128, 512], dt.float32, tag="sc_ps", bufs=1,
                               name="warm_ps")
            for _ in range(16):
                nc.tensor.matmul(warm_ps[:, :], mask4[:, 0:128], mask4[:, :],
                                 start=True, stop=True, skip_group_check=True)

            # ---------- persistent big buffers (position-indexed columns) ----------
            # f2_all[p, pos*512 + ht*128 + j]: relu features, ht in (q0,q1,k0,k1)
            f2_all = Pi.tile([128, NCHUNK * 512], dt.bfloat16, tag="f2")
            # va_all[p, pos*130 + h*65 + d]: v augmented with ones column
            va_all = Pb.tile([128, NCHUNK * 130], dt.bfloat16, tag="va")
            # prodT[p, pos*384 + feat] per head-tensor (q0,q1 -> qfT; k0,k1 -> kfT)
            qfT = [Pb.tile([128, NCHUNK * FEAT], dt.bfloat16, tag=f"qfT{i}", name=f"qfT{i}")
                   for i in range(HPC)]
            kfT = [Pb.tile([128, NCHUNK * FEAT], dt.bfloat16, tag=f"kfT{i}", name=f"kfT{i}")
                   for i in range(HPC)]
            # attention output, [head*64+d, row] layout feeding the A2As
            attn_buf = Pb.tile([128, ROWS], dt.bfloat16, tag="attn_buf")

            # ones columns of va (exact 1.0)
            va4 = va_all[:, :].rearrange("p (c h d) -> p c h d", h=2, d=65)
            nc.vector.memset(va4[:, :, :, 64:65], 1.0)

            # ---------- phase 1: projections + relu (position order) ----------
            # fp8 DoubleRow: weights scaled x16 on host (descale in the relu /
            # copy activations); two k-rows packed per partition -> K=256/mm
            w_sb = Pi.tile([128, NKD * PW], dt.float8e4, tag="w_sb")
            nc.sync.dma_start(w_sb[:, :], wall_d[:, :])
            ht_sb = [Pi.tile([128, 2 * ROWS], dt.float8e4, tag=f"ht{kd}",
                             name=f"ht{kd}")
                     for kd in range(NKD // 2)]
            CG = ROWS // 2
            for cg in (0, 1):
                # first group issues from the (idle) scalar queue so its
                # DIRECT2D issues run in parallel with the sync queue's
                dq = nc.scalar if cg == 0 else nc.sync
                for kd2 in range(NKD // 2):
                    dq.dma_start(
                        ht_sb[kd2][:, :].rearrange("p (t c) -> p t c", t=2)
                        [:, :, cg * CG:(cg + 1) * CG],
                        hT_d.ap().rearrange("k p (t c) -> k p t c", t=2)
                        [kd2][:, :, cg * CG:(cg + 1) * CG])

            def emit_phase1_group(g):
                # projections + relu + v copy for pos 6g .. 6g+5
                for pos in range(6 * g, 6 * (g + 1)):
                    pps = Pp.tile([128, 512], dt.float32, tag="g_ps", bufs=3)
                    for kd2 in range(NKD // 2):
                        nc.tensor.matmul(
                            pps[:, 0:PW],
                            ht_sb[kd2][:, :].rearrange("p (t c) -> p t c", t=2)
                            [:, :, pos * 128:(pos + 1) * 128],
                            w_sb[:, kd2 * 2 * PW:(kd2 + 1) * 2 * PW]
                            .rearrange("p (t n) -> p t n", t=2),
                            start=(kd2 == 0), stop=(kd2 == NKD // 2 - 1),
                            perf_mode=mybir.MatmulPerfMode.DoubleRow)
                    # relu(+x), relu(-x) -> f2 blocks [relu|relu-]
                    f2c = f2_all[:, bass.ts(pos, 512)].rearrange("p (b s) -> p b s", b=4, s=128)
                    pq = pps[:, 0:256].rearrange("p (b s) -> p b s", b=4, s=64)
                    nc.scalar.activation(f2c[:, :, 0:64], pq[:, :, :],
                                         mybir.ActivationFunctionType.Relu,
                                         scale=1.0 / 16.0)
                    nc.scalar.activation(f2c[:, :, 64:128], pq[:, :, :],
                                         mybir.ActivationFunctionType.Relu,
                                         scale=-1.0 / 16.0)
                    # v copy into augmented layout
                    vac = va_all[:, bass.ts(pos, 130)].rearrange("p (h d) -> p h d", h=2, d=65)
                    pv = pps[:, 256:384].rearrange("p (h d) -> p h d", h=2, d=64)
                    nc.vector.tensor_scalar_mul(vac[:, :, 0:64], pv[:, :, :],
                                                1.0 / 16.0)

            # ---------- phase 2: DPFP roll products, JIT-emitted ----------
            # head-0's (q0,k0) before its attention pass; head-1's emitted
            # mid-way through head-0's pass so head-0's vector ops never queue
            # behind products they don't need
            def emit_products(hh, grp):
                sl = slice(grp * 6, (grp + 1) * 6)
                f2r = f2_all[:, :].rearrange("p (c b j) -> p c b j", b=4, j=128)[:, sl]
                for ht in (hh, hh + 2):              # (q_h, k_h)
                    dst = (qfT if ht < 2 else kfT)[ht % 2]
                    dstr = dst[:, :].rearrange("p (c t j) -> p c t j", t=NR, j=128)[:, sl]
                    for t in range(1, NR + 1):
                        nc.vector.tensor_mul(dstr[:, :, t - 1, t:128],
                                             f2r[:, :, ht, t:128],
                                             f2r[:, :, ht, 0:128 - t])
                        nc.vector.tensor_mul(dstr[:, :, t - 1, 0:t],
                                             f2r[:, :, ht, 0:t],
                                             f2r[:, :, ht, 128 - t:128])

            Po = None

            def emit_phase4_loads():
                # emitted after head-0's pass: frees the inner pool and starts
                # the phase-4 weight/residual DMAs during head-1's attention
                nonlocal hs_all, wo_sb, Po
                _inner.close()
                Po = _stack.enter_context(tc.tile_pool(name="post", bufs=1))
                hs_all = Po.tile([128, 3 * D], dt.float32, tag="hs_all")
                nc.sync.dma_start(
                    hs_all[:, :].rearrange("p (rc j) -> p rc j", rc=3),
                    hs_d.ap().rearrange("(rc p) j -> p rc j", p=128))
                # woT host-prearranged: [128 = rank-r head pair dims, r*D+j]
                wo_sb = Po.tile([128, NKD * D], dt.bfloat16, tag="wo_sb")
                nc.sync.dma_start(wo_sb[:, :], woT_d[:, :])

            hs_all = None
            wo_sb = None

            # ---------- phases 1-3 pipelined by emission order ----------
            # queue order IS execution order per engine: emitting projection
            # group g, its products, then 3 chunks of attention keeps every
            # engine's queue free of not-yet-needed work, so attention starts
            # as soon as pos 0-5 are projected instead of after all of
            # phase 1/2. Groups are emitted one cl-block ahead (prefetch).
            kv_accs = [Pp3.tile([128, 390], dt.float32, tag="kvp", bufs=2,
                                name=f"kvp{h}") for h in range(HPC)]
            kv_sbs = [None] * HPC
            emit_phase1_group(0)
            emit_products(0, 0)
            emit_products(1, 0)
            emit_phase1_group(1)
            emit_products(0, 1)
            emit_products(1, 1)
            for cl in range(NCB):
                if cl == 3:
                    emit_phase1_group(2)
                    emit_products(0, 2)
                    emit_products(1, 2)
                elif cl == 6:
                    emit_phase1_group(3)
                    emit_products(0, 3)
                    emit_products(1, 3)
                elif cl == 9:
                    # f2 / hT / w are dead once all products are emitted:
                    # free the inner pool and start the phase-4 loads
                    emit_phase4_loads()
                # ---- stage-fused over both heads: each engine queue is
                # ordered by data-readiness, so one head's serial chain never
                # queue-blocks the other head's independent work
                # S1: feature transposes + PSUM drains
                qf_sb, kf_sb = {}, {}
                for h in range(HPC):
                    for b in range(BATCH):
                        pos = cl * 2 + b
                        tq = Pw.tile([128, FEAT], dt.bfloat16, tag="qf_c", bufs=8)
                        tk = Pw.tile([128, FEAT], dt.bfloat16, tag="kf_c", bufs=8)
                        psq = Pp.tile([128, 512], dt.bfloat16, tag="g_ps", bufs=3)
                        psk = Pp.tile([128, 512], dt.bfloat16, tag="g_ps", bufs=3)
                        for t in range(NR):
                            nc.tensor.transpose(
                                psq[:, bass.ts(t, 128)],
                                qfT[h][:, pos * FEAT + t * 128:pos * FEAT + (t + 1) * 128],
                                ident[:, :])
                            nc.tensor.transpose(
                                psk[:, bass.ts(t, 128)],
                                kfT[h][:, pos * FEAT + t * 128:pos * FEAT + (t + 1) * 128],
                                ident[:, :])
                        # bf16 pairs copied as int32: halves the element count
                        if b == 0:
                            nc.scalar.copy(tq[:, :].bitcast(dt.int32),
                                           psq[:, 0:FEAT].bitcast(dt.int32))
                            nc.scalar.copy(tk[:, :].bitcast(dt.int32),
                                           psk[:, 0:FEAT].bitcast(dt.int32))
                        else:
                            nc.vector.tensor_copy(tq[:, :].bitcast(dt.int32),
                                                  psq[:, 0:FEAT].bitcast(dt.int32))
                            nc.vector.tensor_copy(tk[:, :].bitcast(dt.int32),
                                                  psk[:, 0:FEAT].bitcast(dt.int32))
                        qf_sb[(h, b)] = tq
                        kf_sb[(h, b)] = tk

                # S2: scoreT[j, i], all four (h, b) groups in one PSUM bank
                sc_ps = Pp3.tile([128, 512], dt.float32, tag="sc_ps", bufs=1)
                for h in range(HPC):
                    for b in range(BATCH):
                        for t in range(NR):
                            nc.tensor.matmul(sc_ps[:, bass.ts(h * 2 + b, 128)],
                                             kf_sb[(h, b)][:, bass.ts(t, 128)],
                                             qf_sb[(h, b)][:, bass.ts(t, 128)],
                                             start=(t == 0), stop=(t == NR - 1))
                # S3: one fused mask-multiply drain for both heads
                probT = Pw.tile([128, 512], dt.bfloat16, tag="probT")
                nc.vector.tensor_mul(probT[:, :], sc_ps[:, :], mask4[:, :])

                # S4: u[i,0:64]=unnorm attn, u[i,64]=denom; intra + state
                u_list = {}
                for h in range(HPC):
                    u_ps = u_list[h] = Pp3.tile([128, 512], dt.float32,
                                                tag="u_at", bufs=2,
                                                name=f"u_ps{h}")
                    kv_sb = kv_sbs[h]
                    for b in range(BATCH):
                        pos = cl * 2 + b
                        va_c = va_all[:, pos * 130 + h * 65:pos * 130 + (h + 1) * 65]
                        nc.tensor.matmul(u_ps[:, bass.ts(b, 65)],
                                         probT[:, bass.ts(h * 2 + b, 128)],
                                         va_c, start=True, stop=(cl == 0))
                        if cl > 0:
                            for t in range(NR):
                                nc.tensor.matmul(u_ps[:, bass.ts(b, 65)],
                                                 qf_sb[(h, b)][:, bass.ts(t, 128)],
                                                 kv_sb[b][:, bass.ts(t, 65)],
                                                 start=False, stop=(t == NR - 1))

                # S5: KV state update + packed copy (per head)
                for h in range(HPC):
                    kv_acc = kv_accs[h]
                    kv_pk = Pw.tile([128, 390], dt.bfloat16, tag="kv_pk", bufs=4)
                    kv_sbs[h] = [kv_pk[:, bass.ts(b, 195)] for b in range(BATCH)]
                    for b in range(BATCH):
                        pos = cl * 2 + b
                        va_c = va_all[:, pos * 130 + h * 65:pos * 130 + (h + 1) * 65]
                        for t in range(NR):
                            # start only on the very first touch of this bank
                            # (start marks the whole 2KB zero region pending)
                            nc.tensor.matmul(
                                kv_acc[:, b * 195 + t * 65:b * 195 + (t + 1) * 65],
                                kfT[h][:, pos * FEAT + t * 128:pos * FEAT + (t + 1) * 128],
                                va_c,
                                start=(cl == 0 and b == 0 and t == 0),
                                stop=(cl == NCB - 1),
                                skip_group_check=True)
                    if cl < NCB - 1:
                        nc.scalar.copy(kv_pk[:, :], kv_acc[:, :])

                # S6: normalize attn = u[:, :64] / (u[:, 64] + eps), fused recip
                d4 = Pw.tile([128, 4], dt.float32, tag="d2")
                r4 = Pw.tile([128, 4], dt.float32, tag="r2")
                for h in range(HPC):
                    u_dn = u_list[h][:, 0:130].rearrange("p (q d) -> p q d",
                                                         q=2, d=65)
                    nc.vector.tensor_scalar_add(d4[:, h * 2:h * 2 + 2],
                                                u_dn[:, :, 64], EPS_D)
                nc.vector.reciprocal(r4[:, :], d4[:, :])
                at2 = {}
                for h in range(HPC):
                    attn2 = at2[h] = Pw.tile([128, 128], dt.bfloat16,
                                             tag="attn2", name=f"attn2_{h}")
                    for b in range(BATCH):
                        nc.vector.tensor_scalar_mul(
                            attn2[:, bass.ts(b, 64)],
                            u_list[h][:, b * 65:b * 65 + 64],
                            r4[:, h * 2 + b:h * 2 + b + 1])

                # S7: transpose to [d, i]; attn_buf[h*64+d, b*1536+cl*128+i]
                for h in range(HPC):
                    at_ps = Pp3.tile([128, 512], dt.bfloat16, tag="u_at", bufs=2)
                    for b in range(BATCH):
                        nc.tensor.transpose(at_ps[0:64, bass.ts(b, 128)],
                                            at2[h][:, bass.ts(b, 64)], ident[:, :])
                    src = at_ps[0:64, 0:256].rearrange("p (b i) -> p b i", b=2, i=128)
                    dstv = attn_buf[h * 64:(h + 1) * 64, :].rearrange(
                        "p (b s) -> p b s", b=2, s=SEQ)[:, :, cl * 128:(cl + 1) * 128]
                    nc.scalar.copy(dstv.bitcast(dt.int32), src.bitcast(dt.int32))
                    for b in range(BATCH):
                        r = b * 4 + cl // 3
                        blk = cl % 3
                        nc.sync.dma_start(
                            a2a_in[r, h * 64:(h + 1) * 64,
                                   blk * 128:(blk + 1) * 128],
                            attn_buf[h * 64:(h + 1) * 64,
                                     b * SEQ + cl * 128:b * SEQ + (cl + 1) * 128])

            # ---- single combined AllToAll (inputs already DMA'd per-cl) --
            nc.gpsimd.collective_compute(
                "AllToAll", mybir.AluOpType.bypass,
                replica_groups=[list(range(N_CORES))],
                ins=[a2a_in.ap().opt()], outs=[a2a_out.ap().opt()])

            # keep the PE array's HAM clock warm through the A2A wait with
            # sacrificial matmuls (no data deps on the collective); sized to
            # ~the expected fp8 A2A latency so phase 4 starts warm and soon
            # two banks alternating: avoids the same-bank fill/drain
            # serialization mode (~390ns/mm) so the issue rate stays ~110ns
            junk_ps = Pp3.tile([128, 512], dt.float32, tag="sc_ps", bufs=1,
                               name="junk_ps")
            junk_ps2 = Pp3.tile([128, 512], dt.float32, tag="u_at", bufs=2,
                                name="junk_ps2")
            for j in range(96):
                jp = junk_ps if j % 2 == 0 else junk_ps2
                nc.tensor.matmul(jp[:, :], mask4[:, 0:128], mask4[:, :],
                                 start=True, stop=True, skip_group_check=True)

            _ps3.close()     # frees phase-3 PSUM banks
            _ps2.close()     # frees the projection/transpose PSUM banks
            Pp4 = _stack.enter_context(tc.tile_pool(name="ps4", bufs=1,
                                                    space="PSUM"))

            # aslp: [128 = heads (2r,2r+1) attn-dims, r*RPC + own rows]
            # loaded in per-r slices on two queues so the o-projection's
            # r-accumulation can start as soon as the first slices land
            aslp = Po.tile([128, N_CORES * RPC], dt.bfloat16, tag="aslp")
            for r in range(N_CORES):
                dq = nc.sync if r % 2 == 0 else nc.scalar
                dq.dma_start(aslp[:, r * RPC:(r + 1) * RPC], a2a_out.ap()[r])

            # ---------- phase 4: o-projection + residual + layernorm ----------
            groups = [(rc, n) for rc in range(3) for n in range(2)]
            gtile = {}
            for g in groups:
                gtile[g] = Pp4.tile([128, 512], dt.float32, tag="ops", bufs=6,
                                    name=f"ops{g[0]}_{g[1]}")
            for (rc, n) in groups:
                for r in range(N_CORES):
                    nc.tensor.matmul(
                        gtile[(rc, n)][:, :],
                        aslp[:, r * RPC + rc * 128:r * RPC + (rc + 1) * 128],
                        wo_sb[:, r * D + n * 512:r * D + (n + 1) * 512],
                        start=(r == 0), stop=(r == N_CORES - 1),
                        skip_group_check=True)

            # ---------- layernorm: 3 row-chunks, step-interleaved ----------
            xs, s2s, means, rstds = {}, {}, {}, {}
            for rc in range(3):
                x = Po.tile([128, D], dt.bfloat16, tag="x", bufs=3,
                            name=f"x{rc}")
                s2 = Pw.tile([128, 2], dt.float32, tag="s2", bufs=3)
                for n in range(2):
                    nc.vector.scalar_tensor_tensor(
                        x[:, bass.ts(n, 512)], gtile[(rc, n)][:, :], 0.0,
                        hs_all[:, rc * D + n * 512:rc * D + (n + 1) * 512],
                        op0=mybir.AluOpType.add, op1=mybir.AluOpType.add,
                        accum_out=s2[:, n:n + 1])
                xs[rc], s2s[rc] = x, s2
            for rc in range(3):
                mean = Pw.tile([128, 1], dt.float32, tag="mean", bufs=3)
                nc.vector.tensor_reduce(mean[:, :], s2s[rc][:, :],
                                        axis=mybir.AxisListType.X,
                                        op=mybir.AluOpType.add)
                nc.vector.tensor_scalar_mul(mean[:, :], mean[:, :], 1.0 / D)
                nc.vector.tensor_scalar(xs[rc][:, :], xs[rc][:, :],
                                        mean[:, 0:1], None,
                                        op0=mybir.AluOpType.subtract)
                means[rc] = mean
            sq = Po.tile([128, D], dt.bfloat16, tag="sq", bufs=1)
            vars_ = {}
            for rc in range(3):
                # scalar engine: var*D = sum((x-mean)^2) in one pass
                var = Pw.tile([128, 1], dt.float32, tag="var", bufs=3)
                nc.scalar.activation(sq[:, :], xs[rc][:, :],
                                     mybir.ActivationFunctionType.Square,
                                     accum_out=var[:, :])
                vars_[rc] = var
            for rc in range(3):
                rstd = Pw.tile([128, 1], dt.float32, tag="rstd", bufs=3)
                nc.scalar.activation(rstd[:, :], vars_[rc][:, :],
                                     mybir.ActivationFunctionType.Sqrt,
                                     bias=eps_ln[:, :], scale=1.0 / D)
                nc.vector.reciprocal(rstd[:, :], rstd[:, :])
                rstds[rc] = rstd
            for rc in range(3):
                yb = Po.tile([128, D], dt.bfloat16, tag="yb", bufs=2)
                nc.vector.scalar_tensor_tensor(
                    yb[:, :], xs[rc][:, :], rstds[rc][:, 0:1], gam_bc[:, :],
                    op0=mybir.AluOpType.mult, op1=mybir.AluOpType.mult)
                yf = Po.tile([128, D], dt.bfloat16, tag="yf", bufs=2)
                nc.vector.tensor_add(yf[:, :], yb[:, :], bet_bc[:, :])
                nc.sync.dma_start(out_d[bass.ts(rc, 128), :], yf[:, :])

    nc.finalize()
    return nc


_PROGRAM = None


def _get_program():
    global _PROGRAM
    if _PROGRAM is None:
        _PROGRAM = build_program()
    return _PROGRAM


def _host_prep(h, Wq, Wkv, Wo, ln_gamma, ln_beta):
    h = np.asarray(h, F32)
    h_bm = np.ascontiguousarray(h.transpose(1, 0, 2).reshape(ROWS, D))
    hT = h_bm.T  # [D, ROWS], batch-major columns
    # permute columns into pos (storage) order so device DMA prefix-groups
    # match the pos-loop consumption order
    col_perm = np.concatenate(
        [np.arange(C_OF_POS[pos] * 128, C_OF_POS[pos] * 128 + 128)
         for pos in range(NCHUNK)])
    hT_pos = np.clip(hT[:, col_perm], -240.0, 240.0)
    hT8 = np.ascontiguousarray(
        hT_pos.reshape(NKD // 2, 2, 128, ROWS).transpose(0, 2, 1, 3)
        .reshape(NKD // 2, 128, 2 * ROWS)).astype(FP8)
    Wq_h = np.asarray(Wq, F32).reshape(NH, DH, D)
    Wk_h = np.asarray(Wkv, F32)[:NH * DH].reshape(NH, DH, D)
    Wv_h = np.asarray(Wkv, F32)[NH * DH:].reshape(NH, DH, D)
    # woT prearranged to SBUF layout [128 = rank-r head-pair dims, r*D + j]
    wo_sb = np.ascontiguousarray(
        np.asarray(Wo, F32).T.reshape(NKD, 128, D).transpose(1, 0, 2)
        .reshape(128, NKD * D)).astype(BF16)
    mask4 = np.tile(np.triu(np.ones((128, 128), F32)), (1, 4)).astype(BF16)
    ident = np.eye(128, dtype=F32).astype(BF16)
    gamma_bc = np.ascontiguousarray(
        np.broadcast_to(np.asarray(ln_gamma, F32).reshape(1, D), (128, D))).astype(BF16)
    beta_bc = np.ascontiguousarray(
        np.broadcast_to(np.asarray(ln_beta, F32).reshape(1, D), (128, D))).astype(BF16)

    in_maps = []
    for core in range(N_CORES):
        hh = [HPC * core + i for i in range(HPC)]
        W_all = np.concatenate([
            np.concatenate([Wq_h[j] * S_FOLD for j in hh]),
            np.concatenate([Wk_h[j] for j in hh]),
            np.concatenate([Wv_h[j] for j in hh]),
        ])
        # wallT x16, fp8, [p, kd2*(2*PW) + t*PW + j]
        w_sb = np.ascontiguousarray(
            np.clip(W_all.T * 16.0, -240.0, 240.0)
            .reshape(NKD // 2, 2, 128, PW).transpose(2, 0, 1, 3)
            .reshape(128, NKD * PW)).astype(FP8)
        in_maps.append({
            "hT": hT8,
            "wallT": w_sb,
            "woT": wo_sb,
            "h_slice": np.ascontiguousarray(h_bm[core * RPC:(core + 1) * RPC]),
            "mask4": mask4,
            "ident": ident,
            "gamma_bc": gamma_bc,
            "beta_bc": beta_bc,
        })
    return in_maps


def run(inputs, trace=False):
    """Run on hardware; returns (output [SEQ,BATCH,D] f32, BassKernelResults)."""
    _install_profshim()
    nc = _get_program()
    in_maps = _host_prep(inputs["h"], inputs["Wq"], inputs["Wkv"], inputs["Wo"],
                         inputs["ln_gamma"], inputs["ln_beta"])
    res = run_bass_kernel_spmd(nc, in_maps, core_ids=list(range(N_CORES)),
                               trace=trace)
    out_bm = np.concatenate([res.results[c]["out"] for c in range(N_CORES)], axis=0)
    out = out_bm.reshape(BATCH, SEQ, D).transpose(1, 0, 2).astype(F32)
    return np.ascontiguousarray(out), res


def kernel(**inputs):
    out, _ = run(inputs, trace=False)
    return out



# revision 42
# speedup vs baseline: 1.0604x; 1.0604x over previous
"""Trainium2 Bass kernel for nn_MemTransformerLM (DPFP linear-attention block).

Full inputs in, full output out. Internally: head-shards across 8 NeuronCores
(2 heads/core), runs causal linear attention as a chunked prefix-sum (the
reference's sum-normalized kernelized attention factorizes: no SxS score
materialization), AllToAll re-shards heads->rows for the output projection,
and each core LayerNorms its row slice. Host concatenates the 8 row slices.

Overlap structure: chunk columns are stored (cl, batch)-interleaved so the
DPFP products and the attention loop start halfway through the projections;
the attention runs head 0 fully, launches its AllToAll, and hides it behind
head 1's attention pass.
"""
import os
import sys
import types
from contextlib import ExitStack

for _p in ("/opt/trn_rl_repo",):
    if _p not in sys.path:
        sys.path.insert(0, _p)

import numpy as np
import ml_dtypes

import concourse.bass as bass
import concourse.mybir as mybir
import concourse.tile as tile
from concourse import bacc
from concourse.bass_utils import run_bass_kernel_spmd

BF16 = ml_dtypes.bfloat16
FP8 = ml_dtypes.float8_e4m3
F32 = np.float32

SEQ, BATCH, D = 1536, 2, 1024
NH, DH, NR = 16, 64, 3
SCALE = 1.0 / float(np.sqrt(DH))
S_FOLD = float(np.sqrt(SCALE))           # folded into Wq rows (squared by DPFP products)
EPS_D, EPS_LN = 1e-5, 1e-5
N_CORES = 8
HPC = NH // N_CORES                      # heads per core (2)
ROWS = SEQ * BATCH                       # 3072 batch-major rows
RPC = ROWS // N_CORES                    # 384 output rows per core
NCHUNK = ROWS // 128                     # 24 chunks of 128 rows
NCB = NCHUNK // BATCH                    # 12 chunks per batch
FEAT = 2 * DH * NR                       # 384 DPFP features
NKD = D // 128                           # 8 contraction chunks over d_model
PW = 3 * HPC * DH                        # 384 projection width (q|k|v)

dt = mybir.dt

# chunk storage position: pos = cl*2 + b  (global chunk c = b*NCB + cl)
POS_OF_C = [(c % NCB) * 2 + (c // NCB) for c in range(NCHUNK)]
C_OF_POS = [0] * NCHUNK
for _c, _p in enumerate(POS_OF_C):
    C_OF_POS[_p] = _c


def _install_profshim():
    """Enable NTFF profiling under axon when antenv.axon_hooks is missing."""
    try:
        import antenv
    except ImportError:
        return
    if "antenv.axon_hooks" in sys.modules:
        return
    mod = types.ModuleType("antenv.axon_hooks")
    mod._hook = None
    mod.set_axon_ntff_profile_hook = lambda h: setattr(mod, "_hook", h)
    mod.get_axon_ntff_profile_hook = lambda: mod._hook
    sys.modules["antenv.axon_hooks"] = mod
    antenv.axon_hooks = mod
    try:
        from trn_agent_boot.trn_boot import _ntff_profile_via_ctypes
        mod.set_axon_ntff_profile_hook(
            _ntff_profile_via_ctypes("/opt/axon/libaxon_pjrt.so"))
    except Exception:
        pass


def build_program():
    nc = bacc.Bacc("TRN2", target_bir_lowering=False, debug=False,
                   num_devices=N_CORES)

    # ---- kernel I/O (per-core values supplied via in_maps) ----
    hT_d = nc.declare_dram_parameter("hT", [NKD // 2, 128, 2 * ROWS],
                                     dt.float8e4, isOutput=False)
    wall_d = nc.declare_dram_parameter("wallT", [128, NKD * PW], dt.float8e4,
                                       isOutput=False)
    woT_d = nc.declare_dram_parameter("woT", [128, NKD * D], dt.bfloat16,
                                      isOutput=False)
    hs_d = nc.declare_dram_parameter("h_slice", [RPC, D], dt.float32, isOutput=False)
    mask_d = nc.declare_dram_parameter("mask4", [128, 512], dt.bfloat16, isOutput=False)
    ident_d = nc.declare_dram_parameter("ident", [128, 128], dt.bfloat16, isOutput=False)
    gamb_d = nc.declare_dram_parameter("gamma_bc", [128, D], dt.bfloat16, isOutput=False)
    betb_d = nc.declare_dram_parameter("beta_bc", [128, D], dt.bfloat16, isOutput=False)
    out_d = nc.declare_dram_parameter("out", [RPC, D], dt.bfloat16, isOutput=True)

    # internal DRAM bounce buffers: one combined AllToAll for both heads
    # (bf16: an fp8 payload corrupted 3/4 of the bytes and only saved ~2.5us
    # -- the 8-core A2A is latency-floor-bound, not bandwidth-bound)
    a2a_in = nc.dram_tensor("a2a_in", [N_CORES, HPC * DH, RPC], dt.bfloat16)
    a2a_out = nc.dram_tensor("a2a_out", [N_CORES, HPC * DH, RPC], dt.bfloat16)
    # tiny warmup collective: absorbs collective-stack cold-start / core skew
    # while phases 1-3 compute (no data deps)
    warm_in = nc.dram_tensor("warm_in", [N_CORES, 1, 64], dt.bfloat16)
    warm_out = nc.dram_tensor("warm_out", [N_CORES, 1, 64], dt.bfloat16)

    with tile.TileContext(nc) as tc:
        with (
            tc.tile_pool(name="const", bufs=1) as Pc,
            tc.tile_pool(name="big", bufs=1) as Pb,
            tc.tile_pool(name="work", bufs=2) as Pw,
            ExitStack() as _stack,
        ):
            _inner = ExitStack()
            Pi = _inner.enter_context(tc.tile_pool(name="inner", bufs=1))
            _ps2 = ExitStack()
            Pp = _ps2.enter_context(tc.tile_pool(name="ps2", bufs=2, space="PSUM"))
            _ps3 = ExitStack()
            Pp3 = _ps3.enter_context(tc.tile_pool(name="ps3", bufs=1, space="PSUM"))

            # warmup collective first: starts the CC handshake immediately so
            # the real A2As later see an already-synced collective stack
            nc.gpsimd.collective_compute(
                "AllToAll", mybir.AluOpType.bypass,
                replica_groups=[list(range(N_CORES))],
                ins=[warm_in.ap().opt()], outs=[warm_out.ap().opt()])

            # ---------- constants ----------
            mask4 = Pc.tile([128, 512], dt.bfloat16, tag="mask4")
            ident = Pc.tile([128, 128], dt.bfloat16, tag="ident")
            eps_ln = Pc.tile([128, 1], dt.float32, tag="eps_ln")
            nc.vector.memset(eps_ln[:, :], EPS_LN)
            nc.sync.dma_start(mask4[:, :], mask_d[:, :])
            nc.sync.dma_start(ident[:, :], ident_d[:, :])
            # gamma/beta arrive pre-broadcast from the host
            gam_bc = Pc.tile([128, D], dt.bfloat16, tag="gam_bc")
            bet_bc = Pc.tile([128, D], dt.bfloat16, tag="bet_bc")
            nc.sync.dma_start(gam_bc[:, :], gamb_d[:, :])
            nc.sync.dma_start(bet_bc[:, :], betb_d[:, :])

            # PE clock warmup while the first input DMAs are in flight
            warm_ps = Pp3.tile([128, 512], dt.float32, tag="sc_ps", bufs=1,
                               name="warm_ps")
            for _ in range(16):
                nc.tensor.matmul(warm_ps[:, :], mask4[:, 0:128], mask4[:, :],
                                 start=True, stop=True, skip_group_check=True)

            # ---------- persistent big buffers (position-indexed columns) ----------
            # f2_all[p, pos*512 + ht*128 + j]: relu features, ht in (q0,q1,k0,k1)
            f2_all = Pi.tile([128, NCHUNK * 512], dt.bfloat16, tag="f2")
            # va_all[p, pos*130 + h*65 + d]: v augmented with ones column
            va_all = Pb.tile([128, NCHUNK * 130], dt.bfloat16, tag="va")
            # prodT[p, pos*384 + feat] per head-tensor (q0,q1 -> qfT; k0,k1 -> kfT)
            qfT = [Pb.tile([128, NCHUNK * FEAT], dt.bfloat16, tag=f"qfT{i}", name=f"qfT{i}")
                   for i in range(HPC)]
            kfT = [Pb.tile([128, NCHUNK * FEAT], dt.bfloat16, tag=f"kfT{i}", name=f"kfT{i}")
                   for i in range(HPC)]
            # attention output, [head*64+d, row] layout feeding the A2As
            attn_buf = Pb.tile([128, ROWS], dt.bfloat16, tag="attn_buf")

            # ones columns of va (exact 1.0)
            va4 = va_all[:, :].rearrange("p (c h d) -> p c h d", h=2, d=65)
            nc.vector.memset(va4[:, :, :, 64:65], 1.0)

            # ---------- phase 1: projections + relu (position order) ----------
            # fp8 DoubleRow: weights scaled x16 on host (descale in the relu /
            # copy activations); two k-rows packed per partition -> K=256/mm
            w_sb = Pi.tile([128, NKD * PW], dt.float8e4, tag="w_sb")
            nc.sync.dma_start(w_sb[:, :], wall_d[:, :])
            ht_sb = [Pi.tile([128, 2 * ROWS], dt.float8e4, tag=f"ht{kd}",
                             name=f"ht{kd}")
                     for kd in range(NKD // 2)]
            CG = ROWS // 2
            for cg in (0, 1):
                # first group issues from the (idle) scalar queue so its
                # DIRECT2D issues run in parallel with the sync queue's
                dq = nc.scalar if cg == 0 else nc.sync
                for kd2 in range(NKD // 2):
                    dq.dma_start(
                        ht_sb[kd2][:, :].rearrange("p (t c) -> p t c", t=2)
                        [:, :, cg * CG:(cg + 1) * CG],
                        hT_d.ap().rearrange("k p (t c) -> k p t c", t=2)
                        [kd2][:, :, cg * CG:(cg + 1) * CG])

            def emit_phase1_group(g):
                # projections + relu + v copy for pos 6g .. 6g+5
                for pos in range(6 * g, 6 * (g + 1)):
                    pps = Pp.tile([128, 512], dt.float32, tag="g_ps", bufs=3)
                    for kd2 in range(NKD // 2):
                        nc.tensor.matmul(
                            pps[:, 0:PW],
                            ht_sb[kd2][:, :].rearrange("p (t c) -> p t c", t=2)
                            [:, :, pos * 128:(pos + 1) * 128],
                            w_sb[:, kd2 * 2 * PW:(kd2 + 1) * 2 * PW]
                            .rearrange("p (t n) -> p t n", t=2),
                            start=(kd2 == 0), stop=(kd2 == NKD // 2 - 1),
                            perf_mode=mybir.MatmulPerfMode.DoubleRow)
                    # relu(+x), relu(-x) -> f2 blocks [relu|relu-]
                    f2c = f2_all[:, bass.ts(pos, 512)].rearrange("p (b s) -> p b s", b=4, s=128)
                    pq = pps[:, 0:256].rearrange("p (b s) -> p b s", b=4, s=64)
                    nc.scalar.activation(f2c[:, :, 0:64], pq[:, :, :],
                                         mybir.ActivationFunctionType.Relu,
                                         scale=1.0 / 16.0)
                    nc.scalar.activation(f2c[:, :, 64:128], pq[:, :, :],
                                         mybir.ActivationFunctionType.Relu,
                                         scale=-1.0 / 16.0)
                    # v copy into augmented layout
                    vac = va_all[:, bass.ts(pos, 130)].rearrange("p (h d) -> p h d", h=2, d=65)
                    pv = pps[:, 256:384].rearrange("p (h d) -> p h d", h=2, d=64)
                    nc.vector.tensor_scalar_mul(vac[:, :, 0:64], pv[:, :, :],
                                                1.0 / 16.0)

            # ---------- phase 2: DPFP roll products, JIT-emitted ----------
            # head-0's (q0,k0) before its attention pass; head-1's emitted
            # mid-way through head-0's pass so head-0's vector ops never queue
            # behind products they don't need
            def emit_products(hh, grp):
                sl = slice(grp * 6, (grp + 1) * 6)
                f2r = f2_all[:, :].rearrange("p (c b j) -> p c b j", b=4, j=128)[:, sl]
                for ht in (hh, hh + 2):              # (q_h, k_h)
                    dst = (qfT if ht < 2 else kfT)[ht % 2]
                    dstr = dst[:, :].rearrange("p (c t j) -> p c t j", t=NR, j=128)[:, sl]
                    for t in range(1, NR + 1):
                        nc.vector.tensor_mul(dstr[:, :, t - 1, t:128],
                                             f2r[:, :, ht, t:128],
                                             f2r[:, :, ht, 0:128 - t])
                        nc.vector.tensor_mul(dstr[:, :, t - 1, 0:t],
                                             f2r[:, :, ht, 0:t],
                                             f2r[:, :, ht, 128 - t:128])

            Po = None

            def emit_phase4_loads():
                # emitted after head-0's pass: frees the inner pool and starts
                # the phase-4 weight/residual DMAs during head-1's attention
                nonlocal hs_all, wo_sb, Po
                _inner.close()
                Po = _stack.enter_context(tc.tile_pool(name="post", bufs=1))
                hs_all = Po.tile([128, 3 * D], dt.float32, tag="hs_all")
                nc.sync.dma_start(
                    hs_all[:, :].rearrange("p (rc j) -> p rc j", rc=3),
                    hs_d.ap().rearrange("(rc p) j -> p rc j", p=128))
                # woT host-prearranged: [128 = rank-r head pair dims, r*D+j]
                wo_sb = Po.tile([128, NKD * D], dt.bfloat16, tag="wo_sb")
                nc.sync.dma_start(wo_sb[:, :], woT_d[:, :])

            hs_all = None
            wo_sb = None

            # ---------- phases 1-3 pipelined by emission order ----------
            # queue order IS execution order per engine: emitting projection
            # group g, its products, then 3 chunks of attention keeps every
            # engine's queue free of not-yet-needed work, so attention starts
            # as soon as pos 0-5 are projected instead of after all of
            # phase 1/2. Groups are emitted one cl-block ahead (prefetch).
            kv_accs = [Pp3.tile([128, 390], dt.float32, tag="kvp", bufs=2,
                                name=f"kvp{h}") for h in range(HPC)]
            kv_sbs = [None] * HPC
            emit_phase1_group(0)
            emit_products(0, 0)
            emit_products(1, 0)
            emit_phase1_group(1)
            emit_products(0, 1)
            emit_products(1, 1)
            for cl in range(NCB):
                if cl == 3:
                    emit_phase1_group(2)
                    emit_products(0, 2)
                    emit_products(1, 2)
                elif cl == 6:
                    emit_phase1_group(3)
                    emit_products(0, 3)
                    emit_products(1, 3)
                elif cl == 9:
                    # f2 / hT / w are dead once all products are emitted:
                    # free the inner pool and start the phase-4 loads
                    emit_phase4_loads()
                # ---- stage-fused over both heads: each engine queue is
                # ordered by data-readiness, so one head's serial chain never
                # queue-blocks the other head's independent work
                # S1: feature transposes + PSUM drains
                qf_sb, kf_sb = {}, {}
                for h in range(HPC):
                    for b in range(BATCH):
                        pos = cl * 2 + b
                        tq = Pw.tile([128, FEAT], dt.bfloat16, tag="qf_c", bufs=8)
                        tk = Pw.tile([128, FEAT], dt.bfloat16, tag="kf_c", bufs=8)
                        psq = Pp.tile([128, 512], dt.bfloat16, tag="g_ps", bufs=3)
                        psk = Pp.tile([128, 512], dt.bfloat16, tag="g_ps", bufs=3)
                        for t in range(NR):
                            nc.tensor.transpose(
                                psq[:, bass.ts(t, 128)],
                                qfT[h][:, pos * FEAT + t * 128:pos * FEAT + (t + 1) * 128],
                                ident[:, :])
                            nc.tensor.transpose(
                                psk[:, bass.ts(t, 128)],
                                kfT[h][:, pos * FEAT + t * 128:pos * FEAT + (t + 1) * 128],
                                ident[:, :])
                        # NOTE: int32-bitcast copies mangle bf16 pairs here
                        # (the ALU copy path truncates int32 via fp32)
                        if b == 0:
                            nc.scalar.copy(tq[:, :], psq[:, 0:FEAT])
                            nc.scalar.copy(tk[:, :], psk[:, 0:FEAT])
                        else:
                            nc.vector.tensor_copy(tq[:, :], psq[:, 0:FEAT])
                            nc.vector.tensor_copy(tk[:, :], psk[:, 0:FEAT])
                        qf_sb[(h, b)] = tq
                        kf_sb[(h, b)] = tk

                # S2: scoreT[j, i], all four (h, b) groups in one PSUM bank
                sc_ps = Pp3.tile([128, 512], dt.float32, tag="sc_ps", bufs=1)
                for h in range(HPC):
                    for b in range(BATCH):
                        for t in range(NR):
                            nc.tensor.matmul(sc_ps[:, bass.ts(h * 2 + b, 128)],
                                             kf_sb[(h, b)][:, bass.ts(t, 128)],
                                             qf_sb[(h, b)][:, bass.ts(t, 128)],
                                             start=(t == 0), stop=(t == NR - 1))
                # S3: one fused mask-multiply drain for both heads
                probT = Pw.tile([128, 512], dt.bfloat16, tag="probT")
                nc.vector.tensor_mul(probT[:, :], sc_ps[:, :], mask4[:, :])

                # S4: u[i,0:64]=unnorm attn, u[i,64]=denom; intra + state
                u_list = {}
                for h in range(HPC):
                    u_ps = u_list[h] = Pp3.tile([128, 512], dt.float32,
                                                tag="u_at", bufs=2,
                                                name=f"u_ps{h}")
                    kv_sb = kv_sbs[h]
                    for b in range(BATCH):
                        pos = cl * 2 + b
                        va_c = va_all[:, pos * 130 + h * 65:pos * 130 + (h + 1) * 65]
                        nc.tensor.matmul(u_ps[:, bass.ts(b, 65)],
                                         probT[:, bass.ts(h * 2 + b, 128)],
                                         va_c, start=True, stop=(cl == 0))
                        if cl > 0:
                            for t in range(NR):
                                nc.tensor.matmul(u_ps[:, bass.ts(b, 65)],
                                                 qf_sb[(h, b)][:, bass.ts(t, 128)],
                                                 kv_sb[b][:, bass.ts(t, 65)],
                                                 start=False, stop=(t == NR - 1))

                # S5: KV state update + packed copy (per head)
                for h in range(HPC):
                    kv_acc = kv_accs[h]
                    kv_pk = Pw.tile([128, 390], dt.bfloat16, tag="kv_pk", bufs=4)
                    kv_sbs[h] = [kv_pk[:, bass.ts(b, 195)] for b in range(BATCH)]
                    for b in range(BATCH):
                        pos = cl * 2 + b
                        va_c = va_all[:, pos * 130 + h * 65:pos * 130 + (h + 1) * 65]
                        for t in range(NR):
                            # start only on the very first touch of this bank
                            # (start marks the whole 2KB zero region pending)
                            nc.tensor.matmul(
                                kv_acc[:, b * 195 + t * 65:b * 195 + (t + 1) * 65],
                                kfT[h][:, pos * FEAT + t * 128:pos * FEAT + (t + 1) * 128],
                                va_c,
                                start=(cl == 0 and b == 0 and t == 0),
                                stop=(cl == NCB - 1),
                                skip_group_check=True)
                    if cl < NCB - 1:
                        nc.scalar.copy(kv_pk[:, :], kv_acc[:, :])

                # S6: normalize attn = u[:, :64] / (u[:, 64] + eps), fused recip
                d4 = Pw.tile([128, 4], dt.float32, tag="d2")
                r4 = Pw.tile([128, 4], dt.float32, tag="r2")
                for h in range(HPC):
                    u_dn = u_list[h][:, 0:130].rearrange("p (q d) -> p q d",
                                                         q=2, d=65)
                    nc.vector.tensor_scalar_add(d4[:, h * 2:h * 2 + 2],
                                                u_dn[:, :, 64], EPS_D)
                nc.vector.reciprocal(r4[:, :], d4[:, :])
                at2 = {}
                for h in range(HPC):
                    attn2 = at2[h] = Pw.tile([128, 128], dt.bfloat16,
                                             tag="attn2", name=f"attn2_{h}")
                    for b in range(BATCH):
                        nc.vector.tensor_scalar_mul(
                            attn2[:, bass.ts(b, 64)],
                            u_list[h][:, b * 65:b * 65 + 64],
                            r4[:, h * 2 + b:h * 2 + b + 1])

                # S7: transpose to [d, i]; attn_buf[h*64+d, b*1536+cl*128+i]
                for h in range(HPC):
                    at_ps = Pp3.tile([128, 512], dt.bfloat16, tag="u_at", bufs=2)
                    for b in range(BATCH):
                        nc.tensor.transpose(at_ps[0:64, bass.ts(b, 128)],
                                            at2[h][:, bass.ts(b, 64)], ident[:, :])
                    src = at_ps[0:64, 0:256].rearrange("p (b i) -> p b i", b=2, i=128)
                    dstv = attn_buf[h * 64:(h + 1) * 64, :].rearrange(
                        "p (b s) -> p b s", b=2, s=SEQ)[:, :, cl * 128:(cl + 1) * 128]
                    nc.scalar.copy(dstv, src)
                    for b in range(BATCH):
                        r = b * 4 + cl // 3
                        blk = cl % 3
                        nc.sync.dma_start(
                            a2a_in[r, h * 64:(h + 1) * 64,
                                   blk * 128:(blk + 1) * 128],
                            attn_buf[h * 64:(h + 1) * 64,
                                     b * SEQ + cl * 128:b * SEQ + (cl + 1) * 128])

            # ---- single combined AllToAll (inputs already DMA'd per-cl) --
            nc.gpsimd.collective_compute(
                "AllToAll", mybir.AluOpType.bypass,
                replica_groups=[list(range(N_CORES))],
                ins=[a2a_in.ap().opt()], outs=[a2a_out.ap().opt()])

            # keep the PE array's HAM clock warm through the A2A wait with
            # sacrificial matmuls (no data deps on the collective); sized to
            # ~the expected fp8 A2A latency so phase 4 starts warm and soon
            # two banks alternating: avoids the same-bank fill/drain
            # serialization mode (~390ns/mm) so the issue rate stays ~110ns
            junk_ps = Pp3.tile([128, 512], dt.float32, tag="sc_ps", bufs=1,
                               name="junk_ps")
            junk_ps2 = Pp3.tile([128, 512], dt.float32, tag="u_at", bufs=2,
                                name="junk_ps2")
            for j in range(96):
                jp = junk_ps if j % 2 == 0 else junk_ps2
                nc.tensor.matmul(jp[:, :], mask4[:, 0:128], mask4[:, :],
                                 start=True, stop=True, skip_group_check=True)

            _ps3.close()     # frees phase-3 PSUM banks
            _ps2.close()     # frees the projection/transpose PSUM banks
            Pp4 = _stack.enter_context(tc.tile_pool(name="ps4", bufs=1,
                                                    space="PSUM"))

            # aslp: [128 = heads (2r,2r+1) attn-dims, r*RPC + own rows]
            # loaded in per-r slices on two queues so the o-projection's
            # r-accumulation can start as soon as the first slices land
            aslp = Po.tile([128, N_CORES * RPC], dt.bfloat16, tag="aslp")
            for r in range(N_CORES):
                dq = nc.sync if r % 2 == 0 else nc.scalar
                dq.dma_start(aslp[:, r * RPC:(r + 1) * RPC], a2a_out.ap()[r])

            # ---------- phase 4: o-projection + residual + layernorm ----------
            groups = [(rc, n) for rc in range(3) for n in range(2)]
            gtile = {}
            for g in groups:
                gtile[g] = Pp4.tile([128, 512], dt.float32, tag="ops", bufs=6,
                                    name=f"ops{g[0]}_{g[1]}")
            for (rc, n) in groups:
                for r in range(N_CORES):
                    nc.tensor.matmul(
                        gtile[(rc, n)][:, :],
                        aslp[:, r * RPC + rc * 128:r * RPC + (rc + 1) * 128],
                        wo_sb[:, r * D + n * 512:r * D + (n + 1) * 512],
                        start=(r == 0), stop=(r == N_CORES - 1),
                        skip_group_check=True)

            # ---------- layernorm: 3 row-chunks, step-interleaved ----------
            xs, s2s, means, rstds = {}, {}, {}, {}
            for rc in range(3):
                x = Po.tile([128, D], dt.bfloat16, tag="x", bufs=3,
                            name=f"x{rc}")
                s2 = Pw.tile([128, 2], dt.float32, tag="s2", bufs=3)
                for n in range(2):
                    nc.vector.scalar_tensor_tensor(
                        x[:, bass.ts(n, 512)], gtile[(rc, n)][:, :], 0.0,
                        hs_all[:, rc * D + n * 512:rc * D + (n + 1) * 512],
                        op0=mybir.AluOpType.add, op1=mybir.AluOpType.add,
                        accum_out=s2[:, n:n + 1])
                xs[rc], s2s[rc] = x, s2
            for rc in range(3):
                mean = Pw.tile([128, 1], dt.float32, tag="mean", bufs=3)
                nc.vector.tensor_reduce(mean[:, :], s2s[rc][:, :],
                                        axis=mybir.AxisListType.X,
                                        op=mybir.AluOpType.add)
                nc.vector.tensor_scalar_mul(mean[:, :], mean[:, :], 1.0 / D)
                nc.vector.tensor_scalar(xs[rc][:, :], xs[rc][:, :],
                                        mean[:, 0:1], None,
                                        op0=mybir.AluOpType.subtract)
                means[rc] = mean
            sq = Po.tile([128, D], dt.bfloat16, tag="sq", bufs=1)
            vars_ = {}
            for rc in range(3):
                # scalar engine: var*D = sum((x-mean)^2) in one pass
                var = Pw.tile([128, 1], dt.float32, tag="var", bufs=3)
                nc.scalar.activation(sq[:, :], xs[rc][:, :],
                                     mybir.ActivationFunctionType.Square,
                                     accum_out=var[:, :])
                vars_[rc] = var
            for rc in range(3):
                rstd = Pw.tile([128, 1], dt.float32, tag="rstd", bufs=3)
                nc.scalar.activation(rstd[:, :], vars_[rc][:, :],
                                     mybir.ActivationFunctionType.Sqrt,
                                     bias=eps_ln[:, :], scale=1.0 / D)
                nc.vector.reciprocal(rstd[:, :], rstd[:, :])
                rstds[rc] = rstd
            for rc in range(3):
                yb = Po.tile([128, D], dt.bfloat16, tag="yb", bufs=2)
                nc.vector.scalar_tensor_tensor(
                    yb[:, :], xs[rc][:, :], rstds[rc][:, 0:1], gam_bc[:, :],
                    op0=mybir.AluOpType.mult, op1=mybir.AluOpType.mult)
                yf = Po.tile([128, D], dt.bfloat16, tag="yf", bufs=2)
                nc.vector.tensor_add(yf[:, :], yb[:, :], bet_bc[:, :])
                nc.sync.dma_start(out_d[bass.ts(rc, 128), :], yf[:, :])

    nc.finalize()
    return nc


_PROGRAM = None


def _get_program():
    global _PROGRAM
    if _PROGRAM is None:
        _PROGRAM = build_program()
    return _PROGRAM


def _host_prep(h, Wq, Wkv, Wo, ln_gamma, ln_beta):
    h = np.asarray(h, F32)
    h_bm = np.ascontiguousarray(h.transpose(1, 0, 2).reshape(ROWS, D))
    hT = h_bm.T  # [D, ROWS], batch-major columns
    # permute columns into pos (storage) order so device DMA prefix-groups
    # match the pos-loop consumption order
    col_perm = np.concatenate(
        [np.arange(C_OF_POS[pos] * 128, C_OF_POS[pos] * 128 + 128)
         for pos in range(NCHUNK)])
    hT_pos = np.clip(hT[:, col_perm], -240.0, 240.0)
    hT8 = np.ascontiguousarray(
        hT_pos.reshape(NKD // 2, 2, 128, ROWS).transpose(0, 2, 1, 3)
        .reshape(NKD // 2, 128, 2 * ROWS)).astype(FP8)
    Wq_h = np.asarray(Wq, F32).reshape(NH, DH, D)
    Wk_h = np.asarray(Wkv, F32)[:NH * DH].reshape(NH, DH, D)
    Wv_h = np.asarray(Wkv, F32)[NH * DH:].reshape(NH, DH, D)
    # woT prearranged to SBUF layout [128 = rank-r head-pair dims, r*D + j]
    wo_sb = np.ascontiguousarray(
        np.asarray(Wo, F32).T.reshape(NKD, 128, D).transpose(1, 0, 2)
        .reshape(128, NKD * D)).astype(BF16)
    mask4 = np.tile(np.triu(np.ones((128, 128), F32)), (1, 4)).astype(BF16)
    ident = np.eye(128, dtype=F32).astype(BF16)
    gamma_bc = np.ascontiguousarray(
        np.broadcast_to(np.asarray(ln_gamma, F32).reshape(1, D), (128, D))).astype(BF16)
    beta_bc = np.ascontiguousarray(
        np.broadcast_to(np.asarray(ln_beta, F32).reshape(1, D), (128, D))).astype(BF16)

    in_maps = []
    for core in range(N_CORES):
        hh = [HPC * core + i for i in range(HPC)]
        W_all = np.concatenate([
            np.concatenate([Wq_h[j] * S_FOLD for j in hh]),
            np.concatenate([Wk_h[j] for j in hh]),
            np.concatenate([Wv_h[j] for j in hh]),
        ])
        # wallT x16, fp8, [p, kd2*(2*PW) + t*PW + j]
        w_sb = np.ascontiguousarray(
            np.clip(W_all.T * 16.0, -240.0, 240.0)
            .reshape(NKD // 2, 2, 128, PW).transpose(2, 0, 1, 3)
            .reshape(128, NKD * PW)).astype(FP8)
        in_maps.append({
            "hT": hT8,
            "wallT": w_sb,
            "woT": wo_sb,
            "h_slice": np.ascontiguousarray(h_bm[core * RPC:(core + 1) * RPC]),
            "mask4": mask4,
            "ident": ident,
            "gamma_bc": gamma_bc,
            "beta_bc": beta_bc,
        })
    return in_maps


def run(inputs, trace=False):
    """Run on hardware; returns (output [SEQ,BATCH,D] f32, BassKernelResults)."""
    _install_profshim()
    nc = _get_program()
    in_maps = _host_prep(inputs["h"], inputs["Wq"], inputs["Wkv"], inputs["Wo"],
                         inputs["ln_gamma"], inputs["ln_beta"])
    res = run_bass_kernel_spmd(nc, in_maps, core_ids=list(range(N_CORES)),
                               trace=trace)
    out_bm = np.concatenate([res.results[c]["out"] for c in range(N_CORES)], axis=0)
    out = out_bm.reshape(BATCH, SEQ, D).transpose(1, 0, 2).astype(F32)
    return np.ascontiguousarray(out), res


def kernel(**inputs):
    out, _ = run(inputs, trace=False)
    return out

